# revision 24
# baseline (speedup 1.0000x reference)
"""Trainium2 Bass kernel for a 4-term video/query contrastive loss.

S-orientation, pair-packed design (v4):

  - Device work = the O(B^2 * P * C) query-vs-proposal contrast only:
    scores = qn.T @ vn per video, exp(), and range sums over proposals.
    Norms, top-k, all sentence terms, the O(T*P*C) topk-row sums and the
    final log/mean assembly run on the host in float64.
  - The 64 queries are the stationary operand; video features stream in
    416-column matmuls. Two videos pack one PSUM tile (a: partitions
    0-63, b: 64-127 via matmul col-group base 64), so exp runs all 128
    scalar-engine lanes. ~20 PE instructions per video pair.
  - The host sorts each video's 2080 proposals [iou>0.5 | ==0.5 | <0.5].
    Since n_pos is Binomial(2080,1/2) (sd ~23), the pos/neg boundary
    always falls in group 2 of five 416-column groups (+/-9 sigma
    margin, asserted). Each group's exp is one activation whose
    accum_out yields the group sum for free; group 2 is also written to
    a ship buffer and sent back raw, so the host can split it exactly
    at the boundary. No vector-engine work, no mask matmuls.
"""

import numpy as np
import ml_dtypes

import concourse.bacc as bacc
import concourse.bass as bass
import concourse.tile as tile
from concourse import mybir
from concourse import bass_utils

f32 = mybir.dt.float32
bf16 = mybir.dt.bfloat16
f8 = mybir.dt.float8e4
AFT = mybir.ActivationFunctionType
F8NP = ml_dtypes.float8_e4m3
BF = ml_dtypes.bfloat16

B, C, D = 64, 256, 64
NTRIU = D * (D + 1) // 2   # 2080
NCORES = 8
VB = B // NCORES           # videos per core: 8
NPT = 2
T = B * NPT
NQ = 64                    # queries (stationary cols)
NPAIR = VB // 2
TAU_I = 10.0
SCALE = TAU_I / 256.0
NEG_IOU = 0.5
SPV = NTRIU                # 2080 cols per video half (no pads)
GW = 416                   # group width; 5 groups = 2080
SLABS = (1, 2, 2, 2, 4, 4)
SLABQ = (0, 0, 0, 0, 0, 0)  # 0 = sync HWDGE queue, 1 = scalar HWDGE queue
# group -> (tile index, bank-within-tile); t0 = (g0,g1), t1 = (g3,g4),
# t2 = g2 alone so its exp can write straight to the ship buffer
GT = [(0, 0), (0, 1), (2, 0), (1, 0), (1, 1)]
SHIPG = 2                  # group shipped raw (contains the pos/neg split)


def _build_module():
    nc = bacc.Bacc("TRN2", target_bir_lowering=False, debug=False)

    d_v = nc.dram_tensor("v8", (128, 2 * VB, SPV), f8, kind="ExternalInput")
    d_w = nc.dram_tensor("w8", (128, 2, NQ), f8, kind="ExternalInput")
    d_ac = nc.dram_tensor("accs", (128, NPAIR * 3 + 1), f32,
                          kind="ExternalOutput")
    d_es = nc.dram_tensor("es", (128, NPAIR * GW), bf16,
                          kind="ExternalOutput")

    with tile.TileContext(nc) as tc:
        with (
            tc.tile_pool(name="consts", bufs=1) as cp,
            tc.tile_pool(name="scr", bufs=2) as sp,
            tc.tile_pool(name="gps", bufs=3, space="PSUM") as gp,
            tc.tile_pool(name="gp2", bufs=1, space="PSUM") as gp2,
            tc.tile_pool(name="wrm", bufs=1, space="PSUM") as wp,
        ):
            # preload the ACT Exp table under the input DMAs
            zz = cp.tile([1, 2], f32, tag="zz")
            nc.vector.memset(zz, 0.0)
            zz2 = cp.tile([1, 2], f32, tag="zz2")
            nc.scalar.activation(zz2, zz, AFT.Exp)

            wt = cp.tile([128, 2, NQ], f8, tag="wt")
            nc.scalar.dma_start(wt, d_w[:])
            slab_of = {}
            # block 0 arrives as two column ranges so the first two
            # matmul groups start after 106KB instead of 266KB
            s0 = cp.tile([128, 1, SPV], f8, tag="slab0")
            nc.sync.dma_start(s0[:, :, 0:2 * GW], d_v[:, 0:1, 0:2 * GW])
            nc.sync.dma_start(s0[:, :, 2 * GW:SPV], d_v[:, 0:1, 2 * GW:SPV])
            slab_of[0] = (s0, 0)
            b0 = 1
            for j, nblk in enumerate(SLABS):
                sj = cp.tile([128, nblk, SPV], f8, tag=f"slabx{j}")
                eng = nc.scalar if SLABQ[j] else nc.sync
                eng.dma_start(sj, d_v[:, b0:b0 + nblk, :])
                for m in range(b0, b0 + nblk):
                    slab_of[m] = (sj, m - b0)
                b0 += nblk

            accs_t = cp.tile([128, NPAIR * 3 + 1], f32, tag="accs")
            es_t = cp.tile([128, NPAIR * GW], bf16, tag="es")

            # warm the PE clock (HAM un-throttles after ~3.4us of activity)
            # with dependency-free dummy matmuls during the DMA lead-in
            zmm = cp.tile([128, 128], bf16, tag="zmm")
            nc.vector.memset(zmm, 0.0)
            zps = wp.tile([128, 128], f32, tag="zps")
            for _ in range(20):
                nc.tensor.matmul(zps, zmm, zmm, start=True, stop=True)

            def emit_pair(u, tail):
                t = [gp.tile([128, 2, 512], f32, tag="pg", name=f"pg{u}_{i}")
                     for i in range(2)]
                t.append(gp2.tile([128, GW], f32, tag="pg2", name=f"pg2_{u}",
                                  padded_shape=[128, 512]))

                def mm(vid, r0, h, g):
                    sl, m = slab_of[2 * vid + h]
                    ti, bk = GT[g]
                    dst = t[2][r0:r0 + NQ, :] if ti == 2 \
                        else t[ti][r0:r0 + NQ, bk, 0:GW]
                    nc.tensor.matmul(dst, wt[:, h, :],
                                     sl[:, m, GW * g:GW * g + GW],
                                     start=(h == 0), stop=(h == 1))

                vids = ((2 * u, 0), (2 * u + 1, 64))
                if tail:
                    # video a supply-friendly, video b group-major so the
                    # tail exp chain starts before b's last matmuls finish
                    for h in range(2):
                        for g in range(5):
                            mm(2 * u, 0, h, g)
                    for g in range(5):
                        for h in range(2):
                            mm(2 * u + 1, 64, h, g)
                else:      # video-major: video a runs while b's slab lands
                    for vid, r0 in vids:
                        for h in range(2):
                            for g in range(5):
                                mm(vid, r0, h, g)

                # one exp per psum tile; accum_out = that tile's total,
                # which is all the host needs (a0+a1, a2, a3+a4).
                # The tail pair splits t1 into g3/g4 exps (extra accum col)
                # so each fires as soon as its bank is done.
                def act(src, dst, col):
                    nc.scalar.activation(dst, src, AFT.Exp, scale=SCALE,
                                         accum_out=accs_t[:, col:col + 1])

                sc0 = sp.tile([128, 2, GW], bf16, tag="sc", name=f"sc{u}_0")
                act(t[0][:, :, 0:GW], sc0, 3 * u + 0)
                esd = es_t[:, GW * u:GW * u + GW]
                if tail:
                    act(t[2], esd, 3 * u + 2)
                    sc1 = sp.tile([128, GW], bf16, tag="sc1", name=f"s1{u}a")
                    act(t[1][:, 0, 0:GW], sc1, 3 * u + 1)
                    sc2 = sp.tile([128, GW], bf16, tag="sc1", name=f"s1{u}b")
                    act(t[1][:, 1, 0:GW], sc2, 3 * u + 3)
                else:
                    sc1 = sp.tile([128, 2, GW], bf16, tag="sc",
                                  name=f"sc{u}_1")
                    act(t[1][:, :, 0:GW], sc1, 3 * u + 1)
                    act(t[2], esd, 3 * u + 2)
                eng = nc.sync if tail else nc.gpsimd
                eng.dma_start(d_es[:, GW * u:GW * u + GW], esd)

            for u in range(NPAIR):
                emit_pair(u, tail=(u == NPAIR - 1))
            nc.scalar.dma_start(d_ac[:], accs_t)

    nc.compile()
    return nc


_MODULE = None


def _get_module():
    global _MODULE
    if _MODULE is None:
        _MODULE = _build_module()
    return _MODULE


def kernel(video_feats, query_feats, sents_feats, iou2d, iou2ds, num_targets):
    video_feats = np.asarray(video_feats, np.float32)
    query_feats = np.asarray(query_feats, np.float32)
    sents_feats = np.asarray(sents_feats, np.float32)
    iou2d = np.asarray(iou2d, np.float32)
    iou2ds = np.asarray(iou2ds, np.float32)
    nt = np.asarray(num_targets)
    assert video_feats.shape == (B, C, D, D) and sents_feats.shape == (T, C)
    assert (nt == NPT).all(), "kernel assumes uniform num_targets == 2"

    rows, cols = np.triu_indices(D)
    tri = rows * D + cols

    vtri = video_feats.reshape(B, C, D * D)[:, :, tri]  # (B, C, 2080)
    nrm = np.sqrt(np.einsum('bcp,bcp->bp', vtri, vtri))
    nrm = np.maximum(nrm, 1e-12)
    vn = vtri * (1.0 / nrm)[:, None, :]                 # unit proposals

    iouf = iou2ds.reshape(T, D * D)[:, tri]
    scatter = np.repeat(np.arange(B), NPT)
    amax = np.argmax(iouf, axis=1)
    tvr = vtri[scatter, :, amax]
    tvn = tvr / np.maximum(np.linalg.norm(tvr, axis=1, keepdims=True), 1e-12)
    qn = query_feats / np.maximum(
        np.linalg.norm(query_feats, axis=1, keepdims=True), 1e-12)
    sn = sents_feats / np.maximum(
        np.linalg.norm(sents_feats, axis=1, keepdims=True), 1e-12)

    iou_tri = iou2d.reshape(B, D * D)[:, tri]           # (B, 2080)

    # sort proposals [pos | eq | neg]; the boundary stays inside group 2
    npos = np.empty(B, np.int64)
    vperm = np.empty((B, C, SPV), np.float32)
    for g in range(B):
        io = iou_tri[g]
        pos = np.flatnonzero(io > NEG_IOU)
        eq = np.flatnonzero(io == NEG_IOU)
        neg = np.flatnonzero(io < NEG_IOU)
        npos[g] = len(pos)
        assert GW * 2 < npos[g] and npos[g] + len(eq) <= GW * 3, \
            "pos/neg boundary left group 2"
        vperm[g] = vn[g][:, np.concatenate([pos, eq, neg])]
    v16 = (16.0 * vperm).astype(F8NP)

    # host-side topk-row neg sums (intra-video term)
    negt = np.empty(T, np.float64)
    for g in range(B):
        s = tvn[NPT * g:NPT * g + NPT].astype(np.float32) @ vn[g]
        nm = iou_tri[g] < NEG_IOU
        negt[NPT * g:NPT * g + NPT] = \
            np.exp(TAU_I * s[:, nm].astype(np.float64)).sum(1)

    w8 = np.ascontiguousarray(
        (16.0 * qn).T.astype(F8NP).reshape(2, 128, NQ).transpose(1, 0, 2))

    in_maps = []
    for k in range(NCORES):
        g0 = k * VB
        in_maps.append({
            "v8": np.ascontiguousarray(
                v16[g0:g0 + VB].reshape(2 * VB, 128, SPV)
                .transpose(1, 0, 2)),
            "w8": w8,
        })

    nc = _get_module()
    res = bass_utils.run_bass_kernel_spmd(nc, in_maps,
                                          core_ids=list(range(NCORES)))
    kernel._last = res
    outs = res.results

    # ---- host finalization (float64) ----
    E = np.float64
    valid = np.empty((B, B), E)
    posm = np.empty((B, B), E)
    for k in range(NCORES):
        ac = outs[k]["accs"].astype(E)          # (128, 13)
        es = outs[k]["es"].astype(E)            # (128, 4*416)
        for u in range(NPAIR):
            ncols = 4 if u == NPAIR - 1 else 3
            a3 = ac[:, 3 * u:3 * u + ncols]
            for vloc, r0 in ((2 * u, 0), (2 * u + 1, 64)):
                g = k * VB + vloc
                rr = slice(r0, r0 + NQ)
                raw = es[rr, GW * u:GW * u + GW]
                valid[:, g] = a3[rr].sum(1)
                posm[:, g] = a3[rr, 0] + raw[:, 0:npos[g] - 2 * GW].sum(1)

    qn = qn.astype(E)
    tvn = tvn.astype(E)
    sn = sn.astype(E)

    M1 = tvn @ qn.T
    pos_t = M1[np.arange(T), scatter]
    t1 = -(TAU_I * pos_t - np.log(np.exp(TAU_I * M1).sum(1)))

    negq = valid.sum(1) - posm[np.arange(B), np.arange(B)]
    t2 = -(TAU_I * pos_t - np.log(np.exp(TAU_I * pos_t) + negq[scatter]))

    t3 = []
    for g in range(B):
        tv = tvn[NPT * g:NPT * g + NPT]
        a2 = tv @ tv.T
        for i in range(NPT):
            ns = negt[NPT * g + i]
            for j in range(NPT):
                pd = a2[i, j]
                t3.append(-(TAU_I * pd - np.log(np.exp(TAU_I * pd) + ns)))

    QS = qn @ sn.T
    EQ = np.exp(TAU_I * QS)
    row = EQ.sum(1)
    own = EQ[:, 0::2][np.arange(B), np.arange(B)] \
        + EQ[:, 1::2][np.arange(B), np.arange(B)]
    pos4 = QS[scatter, np.arange(T)]
    t4 = -(TAU_I * pos4
           - np.log(np.exp(TAU_I * pos4) + (row - own)[scatter]))

    return np.stack([t1.mean(), t2.mean(), np.mean(t3),
                     t4.mean()]).astype(np.float32)


# revision 25
# speedup vs baseline: 1.1492x; 1.1492x over previous
"""Trainium2 Bass kernel for a 4-term video/query contrastive loss.

S-orientation, pair-packed design (v4):

  - Device work = the O(B^2 * P * C) query-vs-proposal contrast only:
    scores = qn.T @ vn per video, exp(), and range sums over proposals.
    Norms, top-k, all sentence terms, the O(T*P*C) topk-row sums and the
    final log/mean assembly run on the host in float64.
  - The 64 queries are the stationary operand; video features stream in
    416-column matmuls. Two videos pack one PSUM tile (a: partitions
    0-63, b: 64-127 via matmul col-group base 64), so exp runs all 128
    scalar-engine lanes. ~20 PE instructions per video pair.
  - The host sorts each video's 2080 proposals [iou>0.5 | ==0.5 | <0.5].
    Since n_pos is Binomial(2080,1/2) (sd ~23), the pos/neg boundary
    always falls in group 2 of five 416-column groups (+/-9 sigma
    margin, asserted). Each group's exp is one activation whose
    accum_out yields the group sum for free; group 2 is also written to
    a ship buffer and sent back raw, so the host can split it exactly
    at the boundary. No vector-engine work, no mask matmuls.
"""

import numpy as np
import ml_dtypes

import concourse.bacc as bacc
import concourse.bass as bass
import concourse.tile as tile
from concourse import mybir
from concourse import bass_utils

f32 = mybir.dt.float32
bf16 = mybir.dt.bfloat16
f8 = mybir.dt.float8e4
AFT = mybir.ActivationFunctionType
F8NP = ml_dtypes.float8_e4m3
BF = ml_dtypes.bfloat16

B, C, D = 64, 256, 64
NTRIU = D * (D + 1) // 2   # 2080
NCORES = 8
VB = B // NCORES           # videos per core: 8
NPT = 2
T = B * NPT
NQ = 64                    # queries (stationary cols)
NPAIR = VB // 2
TAU_I = 10.0
SCALE = TAU_I / 256.0
NEG_IOU = 0.5
SPV = NTRIU                # 2080 cols per video half (no pads)
GW = 416                   # group width; 5 groups = 2080
SLABS = (1, 2, 2, 2, 2, 2, 2, 2)
SLABQ = (0, 0, 0, 0, 0, 0, 0, 0)  # 0 = sync HWDGE queue, 1 = scalar HWDGE queue
# group -> (tile index, bank-within-tile); t0 = (g0,g1), t1 = (g3,g4),
# t2 = g2 alone so its exp can write straight to the ship buffer
GT = [(0, 0), (0, 1), (2, 0), (1, 0), (1, 1)]
SHIPG = 2                  # group shipped raw (contains the pos/neg split)


def _build_module():
    nc = bacc.Bacc("TRN2", target_bir_lowering=False, debug=False)

    d_v = nc.dram_tensor("v8", (128, 2 * VB, SPV), f8, kind="ExternalInput")
    d_w = nc.dram_tensor("w8", (128, 2, NQ), f8, kind="ExternalInput")
    d_ac = nc.dram_tensor("accs", (128, NPAIR * 3 + 1), f32,
                          kind="ExternalOutput")
    d_es = nc.dram_tensor("es", (128, NPAIR * GW), bf16,
                          kind="ExternalOutput")

    with tile.TileContext(nc) as tc:
        with (
            tc.tile_pool(name="consts", bufs=1) as cp,
            tc.tile_pool(name="scr", bufs=2) as sp,
            tc.tile_pool(name="gps", bufs=3, space="PSUM") as gp,
            tc.tile_pool(name="gp2", bufs=1, space="PSUM") as gp2,
            tc.tile_pool(name="wrm", bufs=1, space="PSUM") as wp,
        ):
            # preload the ACT Exp table under the input DMAs
            zz = cp.tile([1, 2], f32, tag="zz")
            nc.vector.memset(zz, 0.0)
            zz2 = cp.tile([1, 2], f32, tag="zz2")
            nc.scalar.activation(zz2, zz, AFT.Exp)

            wt = cp.tile([128, 2, NQ], f8, tag="wt")
            nc.scalar.dma_start(wt, d_w[:])
            slab_of = {}
            # block 0 arrives as two column ranges so the first two
            # matmul groups start after 106KB instead of 266KB
            s0 = cp.tile([128, 1, SPV], f8, tag="slab0")
            nc.sync.dma_start(s0[:, :, 0:2 * GW], d_v[:, 0:1, 0:2 * GW])
            nc.sync.dma_start(s0[:, :, 2 * GW:SPV], d_v[:, 0:1, 2 * GW:SPV])
            slab_of[0] = (s0, 0)
            b0 = 1
            for j, nblk in enumerate(SLABS):
                sj = cp.tile([128, nblk, SPV], f8, tag=f"slabx{j}")
                eng = nc.scalar if SLABQ[j] else nc.sync
                eng.dma_start(sj, d_v[:, b0:b0 + nblk, :])
                for m in range(b0, b0 + nblk):
                    slab_of[m] = (sj, m - b0)
                b0 += nblk

            accs_t = cp.tile([128, NPAIR * 3 + 1], f32, tag="accs")
            es_t = cp.tile([128, NPAIR * GW], bf16, tag="es")

            # warm the PE clock (HAM un-throttles after ~3.4us of activity)
            # with dependency-free dummy matmuls during the DMA lead-in
            zmm = cp.tile([128, 128], bf16, tag="zmm")
            nc.vector.memset(zmm, 0.0)
            zps = wp.tile([128, 128], f32, tag="zps")
            for _ in range(20):
                nc.tensor.matmul(zps, zmm, zmm, start=True, stop=True)

            def emit_pair(u, tail):
                t = [gp.tile([128, 2, 512], f32, tag="pg", name=f"pg{u}_{i}")
                     for i in range(2)]
                t.append(gp2.tile([128, GW], f32, tag="pg2", name=f"pg2_{u}",
                                  padded_shape=[128, 512]))

                def mm(vid, r0, h, g):
                    sl, m = slab_of[2 * vid + h]
                    ti, bk = GT[g]
                    dst = t[2][r0:r0 + NQ, :] if ti == 2 \
                        else t[ti][r0:r0 + NQ, bk, 0:GW]
                    nc.tensor.matmul(dst, wt[:, h, :],
                                     sl[:, m, GW * g:GW * g + GW],
                                     start=(h == 0), stop=(h == 1))

                vids = ((2 * u, 0), (2 * u + 1, 64))
                if tail:
                    # video a supply-friendly, video b group-major so the
                    # tail exp chain starts before b's last matmuls finish
                    for h in range(2):
                        for g in range(5):
                            mm(2 * u, 0, h, g)
                    for g in range(5):
                        for h in range(2):
                            mm(2 * u + 1, 64, h, g)
                else:      # video-major: video a runs while b's slab lands
                    for vid, r0 in vids:
                        for h in range(2):
                            for g in range(5):
                                mm(vid, r0, h, g)

                # one exp per psum tile; accum_out = that tile's total,
                # which is all the host needs (a0+a1, a2, a3+a4).
                # The tail pair splits t1 into g3/g4 exps (extra accum col)
                # so each fires as soon as its bank is done.
                def act(src, dst, col):
                    nc.scalar.activation(dst, src, AFT.Exp, scale=SCALE,
                                         accum_out=accs_t[:, col:col + 1])

                sc0 = sp.tile([128, 2, GW], bf16, tag="sc", name=f"sc{u}_0")
                act(t[0][:, :, 0:GW], sc0, 3 * u + 0)
                esd = es_t[:, GW * u:GW * u + GW]
                if tail:
                    act(t[2], esd, 3 * u + 2)
                    sc1 = sp.tile([128, GW], bf16, tag="sc1", name=f"s1{u}a")
                    act(t[1][:, 0, 0:GW], sc1, 3 * u + 1)
                    sc2 = sp.tile([128, GW], bf16, tag="sc1", name=f"s1{u}b")
                    act(t[1][:, 1, 0:GW], sc2, 3 * u + 3)
                else:
                    sc1 = sp.tile([128, 2, GW], bf16, tag="sc",
                                  name=f"sc{u}_1")
                    act(t[1][:, :, 0:GW], sc1, 3 * u + 1)
                    act(t[2], esd, 3 * u + 2)
                eng = nc.sync if tail else nc.gpsimd
                eng.dma_start(d_es[:, GW * u:GW * u + GW], esd)

            for u in range(NPAIR):
                emit_pair(u, tail=(u == NPAIR - 1))
            nc.scalar.dma_start(d_ac[:], accs_t)

    nc.compile()
    return nc


_MODULE = None


def _get_module():
    global _MODULE
    if _MODULE is None:
        _MODULE = _build_module()
    return _MODULE


def kernel(video_feats, query_feats, sents_feats, iou2d, iou2ds, num_targets):
    video_feats = np.asarray(video_feats, np.float32)
    query_feats = np.asarray(query_feats, np.float32)
    sents_feats = np.asarray(sents_feats, np.float32)
    iou2d = np.asarray(iou2d, np.float32)
    iou2ds = np.asarray(iou2ds, np.float32)
    nt = np.asarray(num_targets)
    assert video_feats.shape == (B, C, D, D) and sents_feats.shape == (T, C)
    assert (nt == NPT).all(), "kernel assumes uniform num_targets == 2"

    rows, cols = np.triu_indices(D)
    tri = rows * D + cols

    vtri = video_feats.reshape(B, C, D * D)[:, :, tri]  # (B, C, 2080)
    nrm = np.sqrt(np.einsum('bcp,bcp->bp', vtri, vtri))
    nrm = np.maximum(nrm, 1e-12)
    vn = vtri * (1.0 / nrm)[:, None, :]                 # unit proposals

    iouf = iou2ds.reshape(T, D * D)[:, tri]
    scatter = np.repeat(np.arange(B), NPT)
    amax = np.argmax(iouf, axis=1)
    tvr = vtri[scatter, :, amax]
    tvn = tvr / np.maximum(np.linalg.norm(tvr, axis=1, keepdims=True), 1e-12)
    qn = query_feats / np.maximum(
        np.linalg.norm(query_feats, axis=1, keepdims=True), 1e-12)
    sn = sents_feats / np.maximum(
        np.linalg.norm(sents_feats, axis=1, keepdims=True), 1e-12)

    iou_tri = iou2d.reshape(B, D * D)[:, tri]           # (B, 2080)

    # sort proposals [pos | eq | neg]; the boundary stays inside group 2
    npos = np.empty(B, np.int64)
    vperm = np.empty((B, C, SPV), np.float32)
    for g in range(B):
        io = iou_tri[g]
        pos = np.flatnonzero(io > NEG_IOU)
        eq = np.flatnonzero(io == NEG_IOU)
        neg = np.flatnonzero(io < NEG_IOU)
        npos[g] = len(pos)
        assert GW * 2 < npos[g] and npos[g] + len(eq) <= GW * 3, \
            "pos/neg boundary left group 2"
        vperm[g] = vn[g][:, np.concatenate([pos, eq, neg])]
    v16 = (16.0 * vperm).astype(F8NP)

    # host-side topk-row neg sums (intra-video term)
    negt = np.empty(T, np.float64)
    for g in range(B):
        s = tvn[NPT * g:NPT * g + NPT].astype(np.float32) @ vn[g]
        nm = iou_tri[g] < NEG_IOU
        negt[NPT * g:NPT * g + NPT] = \
            np.exp(TAU_I * s[:, nm].astype(np.float64)).sum(1)

    w8 = np.ascontiguousarray(
        (16.0 * qn).T.astype(F8NP).reshape(2, 128, NQ).transpose(1, 0, 2))

    in_maps = []
    for k in range(NCORES):
        g0 = k * VB
        in_maps.append({
            "v8": np.ascontiguousarray(
                v16[g0:g0 + VB].reshape(2 * VB, 128, SPV)
                .transpose(1, 0, 2)),
            "w8": w8,
        })

    nc = _get_module()
    res = bass_utils.run_bass_kernel_spmd(nc, in_maps,
                                          core_ids=list(range(NCORES)))
    kernel._last = res
    outs = res.results

    # ---- host finalization (float64) ----
    E = np.float64
    valid = np.empty((B, B), E)
    posm = np.empty((B, B), E)
    for k in range(NCORES):
        ac = outs[k]["accs"].astype(E)          # (128, 13)
        es = outs[k]["es"].astype(E)            # (128, 4*416)
        for u in range(NPAIR):
            ncols = 4 if u == NPAIR - 1 else 3
            a3 = ac[:, 3 * u:3 * u + ncols]
            for vloc, r0 in ((2 * u, 0), (2 * u + 1, 64)):
                g = k * VB + vloc
                rr = slice(r0, r0 + NQ)
                raw = es[rr, GW * u:GW * u + GW]
                valid[:, g] = a3[rr].sum(1)
                posm[:, g] = a3[rr, 0] + raw[:, 0:npos[g] - 2 * GW].sum(1)

    qn = qn.astype(E)
    tvn = tvn.astype(E)
    sn = sn.astype(E)

    M1 = tvn @ qn.T
    pos_t = M1[np.arange(T), scatter]
    t1 = -(TAU_I * pos_t - np.log(np.exp(TAU_I * M1).sum(1)))

    negq = valid.sum(1) - posm[np.arange(B), np.arange(B)]
    t2 = -(TAU_I * pos_t - np.log(np.exp(TAU_I * pos_t) + negq[scatter]))

    t3 = []
    for g in range(B):
        tv = tvn[NPT * g:NPT * g + NPT]
        a2 = tv @ tv.T
        for i in range(NPT):
            ns = negt[NPT * g + i]
            for j in range(NPT):
                pd = a2[i, j]
                t3.append(-(TAU_I * pd - np.log(np.exp(TAU_I * pd) + ns)))

    QS = qn @ sn.T
    EQ = np.exp(TAU_I * QS)
    row = EQ.sum(1)
    own = EQ[:, 0::2][np.arange(B), np.arange(B)] \
        + EQ[:, 1::2][np.arange(B), np.arange(B)]
    pos4 = QS[scatter, np.arange(T)]
    t4 = -(TAU_I * pos4
           - np.log(np.exp(TAU_I * pos4) + (row - own)[scatter]))

    return np.stack([t1.mean(), t2.mean(), np.mean(t3),
                     t4.mean()]).astype(np.float32)
